# revision 14
# baseline (speedup 1.0000x reference)
"""Trainium2 Bass kernel for nn_DecoderRNN (GRU + gumbel-softmax sampling decoder).

Strategy:
  - Vocab (50257 -> padded 53248) is sharded 8 ways; each core holds its
    Wd / embedding shard resident in SBUF as bf16.
  - Per step: GRU gates via matmuls into a [32, 2048] PSUM (z|r|xn|hn),
    logits via col-tiled matmuls over the vocab shard (4 groups of 1664),
    exp via ScalarE (with free-dim accum for the softmax normalizer),
    PE transposes to reach vocab-major layout, embedding matmul produces
    unnormalized x-partials, and one AllReduce of [y | s] (32x513 f32)
    combines shards.  Division by the global normalizer happens after the
    AllReduce on every core (replicated small state).
  - All RNG (dropout mask + gumbel noise) is precomputed on host with JAX
    threefry (bit-exact vs the reference) and streamed per step.
  - Cross-partition reductions use a selector matmul on PE (elementwise ops
    must be partition-aligned on trn2).
"""

import numpy as np
import ml_dtypes
import jax

import concourse.bacc as bacc
import concourse.bass as bass
import concourse.mybir as mybir
import concourse.tile as tile
from concourse.bass_utils import run_bass_kernel_spmd

F32 = mybir.dt.float32
BF16 = mybir.dt.bfloat16
AF = mybir.ActivationFunctionType
OP = mybir.AluOpType

B, RNN, EMB, VOCAB, LEN = 32, 512, 512, 50257, 64
NCORES = 8
GN = 1664               # vocab columns per PE column-group
VC = 4 * GN             # 6656 vocab columns per core
VPAD = NCORES * VC      # 53248
NQ = VC // 128          # 52 transposed vocab chunks per core
KEEP = 0.9
EPS = 1e-20
NEG = -1.0e30

_BUILD_CACHE = {}


def _build_program(steps=LEN):
    """Build the SPMD Bass program (identical for all cores)."""
    nc = bacc.Bacc("TRN2", target_bir_lowering=False, debug=False, num_devices=NCORES)

    # ---- DRAM I/O ----
    i_wxh = nc.dram_tensor("i_wxh", [128, 8 * 1536], BF16, kind="ExternalInput").ap()
    i_wd = nc.dram_tensor("i_wd", [128, 4 * VC], BF16, kind="ExternalInput").ap()
    i_emb = nc.dram_tensor("i_emb", [128, NQ * 512], BF16, kind="ExternalInput").ap()
    i_g = nc.dram_tensor("i_g", [steps, 128, GN], F32, kind="ExternalInput").ap()
    i_mask = nc.dram_tensor("i_mask", [steps, 128, 128], F32, kind="ExternalInput").ap()
    i_h0 = nc.dram_tensor("i_h0", [B, RNN], F32, kind="ExternalInput").ap()
    i_h0T = nc.dram_tensor("i_h0T", [128, 128], BF16, kind="ExternalInput").ap()
    i_x0T = nc.dram_tensor("i_x0T", [128, 128], BF16, kind="ExternalInput").ap()
    i_ident = nc.dram_tensor("i_ident", [128, 128], F32, kind="ExternalInput").ap()
    i_sel = nc.dram_tensor("i_sel", [128, 32], F32, kind="ExternalInput").ap()
    i_gamma = nc.dram_tensor("i_gamma", [128, 1], F32, kind="ExternalInput").ap()
    o_hseq = nc.dram_tensor("o_hseq", [B, steps, RNN], F32, kind="ExternalOutput").ap()
    o_logits = nc.dram_tensor("o_logits", [B, steps, VC], F32, kind="ExternalOutput").ap()
    o_warm = nc.dram_tensor("o_warm", [B, 8], F32, kind="ExternalOutput").ap()

    with tile.TileContext(nc) as tc:
        with (
            tc.tile_pool(name="const", bufs=1) as cpool,
            tc.tile_pool(name="gbuf", bufs=2) as gpool,
            tc.tile_pool(name="mbuf", bufs=2) as mpool,
            tc.tile_pool(name="work", bufs=2) as wpool,
            tc.tile_pool(name="small", bufs=2) as spool,
            tc.tile_pool(name="ps_big", bufs=1, space="PSUM") as ps_big,
            tc.tile_pool(name="ps_tp", bufs=2, space="PSUM") as ps_tp,
            tc.tile_pool(name="ps_y", bufs=1, space="PSUM") as ps_y,
            tc.tile_pool(name="ps_ys", bufs=1, space="PSUM") as ps_ys,
            tc.tile_pool(name="dram", bufs=2, space="DRAM") as dpool,
        ):
            # ---- resident constants ----
            wxh_sb = cpool.tile([128, 8 * 1536], BF16)
            for q in range(4):
                nc.sync.dma_start(
                    wxh_sb[:, q * 3072 : (q + 1) * 3072],
                    i_wxh[:, q * 3072 : (q + 1) * 3072],
                )
            wd_sb = cpool.tile([128, 4 * VC], BF16)
            for q in range(4):
                nc.sync.dma_start(
                    wd_sb[:, q * VC : (q + 1) * VC], i_wd[:, q * VC : (q + 1) * VC]
                )
            emb_sb = cpool.tile([128, NQ * 512], BF16)
            for q in range(4):
                w = NQ * 512 // 4
                nc.sync.dma_start(
                    emb_sb[:, q * w : (q + 1) * w], i_emb[:, q * w : (q + 1) * w]
                )
            ident_sb = cpool.tile([128, 128], F32)
            nc.sync.dma_start(ident_sb[:], i_ident[:])
            sel_sb = cpool.tile([128, 32], F32)
            nc.sync.dma_start(sel_sb[:], i_sel[:])
            gamma_sb = cpool.tile([128, 1], F32)
            nc.sync.dma_start(gamma_sb[:], i_gamma[:])

            # ---- state tiles (step 0) ----
            h_prev = spool.tile([B, RNN], F32, tag="h", bufs=3)
            nc.sync.dma_start(h_prev[:], i_h0[:])
            hT = spool.tile([128, 128], BF16, tag="hT", bufs=3)
            nc.sync.dma_start(hT[:], i_h0T[:])
            xT = spool.tile([128, 128], BF16, tag="xT", bufs=3)
            nc.sync.dma_start(xT[:], i_x0T[:])

            for t in range(steps):
                last = t == steps - 1
                # -------- stream per-step randomness --------
                m_t = mpool.tile([128, 128], F32, tag="m")
                nc.sync.dma_start(m_t[:], i_mask[t])
                if not last:
                    g_t = gpool.tile([128, GN], F32, tag="g")
                    nc.sync.dma_start(g_t[:], i_g[t])

                # -------- emit h_t (pre-step hidden) --------
                nc.sync.dma_start(o_hseq[:, t, :], h_prev[:])

                # -------- GRU gates: psum cols [z | r | xn | hn] --------
                gp = ps_big.tile([B, 2048], F32, tag="big")
                for k in range(4):  # h-dependent first (ready before x arrives)
                    lhs = hT[:, k * 32 : (k + 1) * 32]
                    wh = wxh_sb[:, (4 + k) * 1536 :]
                    st = k == 0
                    nc.tensor.matmul(gp[:, 0:512], lhs, wh[:, 0:512], start=st, stop=False)
                    nc.tensor.matmul(gp[:, 512:1024], lhs, wh[:, 512:1024], start=st, stop=False)
                    nc.tensor.matmul(
                        gp[:, 1536:2048], lhs, wh[:, 1024:1536], start=st, stop=(k == 3)
                    )
                for k in range(4):
                    lhs = xT[:, k * 32 : (k + 1) * 32]
                    wx = wxh_sb[:, k * 1536 :]
                    sp = k == 3
                    nc.tensor.matmul(gp[:, 0:512], lhs, wx[:, 0:512], start=False, stop=sp)
                    nc.tensor.matmul(gp[:, 512:1024], lhs, wx[:, 512:1024], start=False, stop=sp)
                    nc.tensor.matmul(
                        gp[:, 1024:1536], lhs, wx[:, 1024:1536], start=(k == 0), stop=sp
                    )

                # -------- gate nonlinearities (all partition-aligned @0:32) ----
                # sigmoid(u) = 0.5*tanh(u/2) + 0.5 (keeps ACT in exp_and_others)
                # tanh(u/2) once; sigmoid folded downstream via (t+1) forms
                zr_t = spool.tile([B, 1024], F32, tag="zr")
                nc.scalar.activation(zr_t[:], gp[:, 0:1024], AF.Tanh, scale=0.5)
                # r*hn = (t_r + 1) * hn'   (Wh_n pre-scaled by 0.5 on host)
                rh = spool.tile([B, 512], F32, tag="rh")
                nc.vector.scalar_tensor_tensor(
                    out=rh[:], in0=zr_t[:, 512:1024], scalar=1.0,
                    in1=gp[:, 1536:2048], op0=OP.add, op1=OP.mult,
                )
                nc.vector.tensor_tensor(
                    out=rh[:], in0=rh[:], in1=gp[:, 1024:1536], op=OP.add
                )
                n_t = rh
                nc.scalar.activation(n_t[:], rh[:], AF.Tanh)
                # h_new = n + z*(h-n) = n + 0.5*((t_z+1)*(h-n)); DVE/GPSIMD halves
                d_t = spool.tile([B, 512], F32, tag="d")
                h_new = spool.tile([B, 512], F32, tag="h", bufs=3)
                HH = 256
                nc.vector.tensor_tensor(
                    out=d_t[:, 0:HH], in0=h_prev[:, 0:HH], in1=n_t[:, 0:HH], op=OP.subtract
                )
                nc.gpsimd.tensor_tensor(
                    out=d_t[:, HH:512], in0=h_prev[:, HH:512], in1=n_t[:, HH:512],
                    op=OP.subtract,
                )
                nc.vector.scalar_tensor_tensor(
                    out=d_t[:], in0=zr_t[:, 0:512], scalar=1.0,
                    in1=d_t[:], op0=OP.add, op1=OP.mult,
                )
                nc.vector.scalar_tensor_tensor(
                    out=h_new[:], in0=d_t[:], scalar=0.5,
                    in1=n_t[:], op0=OP.mult, op1=OP.add,
                )

                # -------- transpose h_new; apply dropout mask --------
                tph = ps_tp.tile([128, 512], F32, tag="tp")
                for k in range(4):
                    nc.tensor.transpose(
                        tph[:, k * 32 : (k + 1) * 32],
                        h_new[:, k * 128 : (k + 1) * 128],
                        ident_sb[0:32, 0:32],
                    )
                hT = spool.tile([128, 128], BF16, tag="hT", bufs=3)
                nc.scalar.copy(hT[:], tph[:, 0:128])
                outT = spool.tile([128, 128], BF16, tag="outT")
                nc.vector.tensor_tensor(
                    out=outT[:], in0=tph[:, 0:128], in1=m_t[:], op=OP.mult
                )

                # -------- logits matmuls (col-tiled over 4 vocab groups) -----
                lp = ps_big.tile([128, 2048], F32, tag="big")
                for off, nn_ in ((0, 512), (512, 512), (1024, 512), (1536, 128)):
                    for k in range(4):
                        for j in range(4):
                            nc.tensor.matmul(
                                lp[32 * j : 32 * (j + 1), off : off + nn_],
                                outT[:, k * 32 : (k + 1) * 32],
                                wd_sb[:, k * VC + j * GN + off : k * VC + j * GN + off + nn_],
                                start=(k == 0), stop=(k == 3),
                                tile_position=(0, 32 * j),
                                skip_group_check=True,
                            )

                # -------- softmax numerator first (keeps ACT critical path
                # short); logits output copy happens after, off critical path.
                logits_sb = wpool.tile([128, GN], F32, tag="lsb", bufs=1)
                if not last:
                    exp_sb = wpool.tile([128, GN], F32, tag="exp", bufs=1)
                    s_parts = spool.tile([128, 4], F32, tag="sp")
                    for si, (off, nn_) in enumerate(((0, 512), (512, 512), (1024, 640))):
                        nc.vector.tensor_tensor(
                            out=g_t[:, off : off + nn_],
                            in0=lp[:, off : off + nn_],
                            in1=g_t[:, off : off + nn_],
                            op=OP.add,
                        )
                        nc.scalar.activation(
                            exp_sb[:, off : off + nn_],
                            g_t[:, off : off + nn_],
                            AF.Exp,
                            accum_out=s_parts[:, si : si + 1],
                        )
                # -------- logits output: psum * gamma -> DRAM --------
                nc.scalar.activation(
                    logits_sb[:], lp[:, 0:GN], AF.Copy, bias=0.0, scale=gamma_sb[:, 0:1]
                )
                for j in range(4):
                    nc.sync.dma_start(
                        o_logits[:, t, j * GN : (j + 1) * GN],
                        logits_sb[32 * j : 32 * (j + 1), :],
                    )

                if last:
                    h_prev = h_new
                    continue

                # -------- transpose exp to vocab-major (13 chunks of 128) ----
                expT = wpool.tile([128, NQ * 32], BF16, tag="expT", bufs=1)
                for r in range(4):
                    cs = range(4 * r, min(4 * r + 4, 13))
                    nch = len(cs)
                    te = ps_tp.tile([128, 512], F32, tag="tp")
                    for ci, c in enumerate(cs):
                        nc.tensor.transpose(
                            te[:, ci * 128 : (ci + 1) * 128],
                            exp_sb[:, c * 128 : (c + 1) * 128],
                            ident_sb[:],
                        )
                    src = te[:, 0 : nch * 128].rearrange(
                        "p (c j bb) -> p c j bb", c=nch, j=4, bb=32
                    )
                    dst = expT[:].rearrange("p (j c bb) -> p c j bb", j=4, c=13, bb=32)[
                        :, 4 * r : 4 * r + nch, :, :
                    ]
                    if r % 2 == 0:
                        nc.vector.tensor_copy(dst, src)
                    else:
                        nc.scalar.copy(dst, src)

                # -------- embedding matmul (partials per column-group) -------
                yp = ps_y.tile([128, 512], F32, tag="yp")
                for c in range(13):
                    for j in range(4):
                        q = 13 * j + c
                        nc.tensor.matmul(
                            yp[32 * j : 32 * (j + 1), :],
                            expT[:, q * 32 : (q + 1) * 32],
                            emb_sb[:, q * 512 : (q + 1) * 512],
                            start=(c == 0), stop=(c == 12),
                            tile_position=(0, 32 * j),
                            skip_group_check=True,
                        )

                # -------- fold partials across partition blocks (PE) --------
                y_sb = spool.tile([128, 512], F32, tag="ysb")
                nc.scalar.copy(y_sb[:], yp[:])
                ys = ps_ys.tile([B, 512], F32, tag="ys")
                nc.tensor.matmul(ys[:], sel_sb[:], y_sb[:], start=True, stop=True)
                xchg = spool.tile([B, 513], F32, tag="xchg")
                nc.vector.tensor_copy(xchg[:, 0:512], ys[:])
                # s: gather the 128x3 partials into batch-partition layout
                s_gather = spool.tile([B, 12], F32, tag="sg")
                for j in range(4):
                    nc.sync.dma_start(
                        s_gather[:, j * 3 : (j + 1) * 3],
                        s_parts[32 * j : 32 * (j + 1), 0:3],
                    )
                nc.vector.reduce_sum(
                    xchg[:, 512:513], s_gather[:], axis=mybir.AxisListType.X
                )

                # -------- AllGather of per-core [y | s]; reduce on PE -----
                cc_in = dpool.tile([B, 513], F32, tag="ccin")
                cc_out = dpool.tile(
                    [NCORES * B, 513], F32, tag="ccout", addr_space="Shared"
                )
                nc.sync.dma_start(cc_in[:], xchg[:])
                nc.gpsimd.collective_compute(
                    "AllGather",
                    OP.bypass,
                    replica_groups=[list(range(NCORES))],
                    ins=[cc_in[:]],
                    outs=[cc_out[:]],
                )

                # PE-warming dummies spanning the collective (HAM stays 8/8)
                warm = ps_ys.tile([B, 512], F32, tag="ys")
                for i in range(28):
                    nc.tensor.matmul(
                        warm[:], sel_sb[:], logits_sb[:, 0:512],
                        start=(i == 0), stop=(i == 27),
                    )
                warm_sb = spool.tile([B, 8], F32, tag="wsb")
                nc.vector.tensor_copy(warm_sb[:], warm[:, 0:8])
                nc.sync.dma_start(o_warm[:], warm_sb[:])

                agg = wpool.tile([128, 2 * 513], F32, tag="agg", bufs=1)
                nc.sync.dma_start(agg[:, 0:513], cc_out[0:128, :])
                nc.sync.dma_start(agg[:, 513:1026], cc_out[128:256, :])
                ysg = ps_ys.tile([B, 512], F32, tag="ys")
                nc.tensor.matmul(
                    ysg[:], sel_sb[:], agg[:, 0:512], start=True, stop=False
                )
                nc.tensor.matmul(
                    ysg[:], sel_sb[:], agg[:, 513:1025], start=False, stop=True
                )
                s_g = spool.tile([B, 8], F32, tag="sg2")
                for j in range(4):
                    nc.sync.dma_start(
                        s_g[:, 2 * j : 2 * j + 2],
                        agg[32 * j : 32 * (j + 1), 512::513],
                    )
                s_tot = spool.tile([B, 1], F32, tag="stot")
                nc.vector.reduce_sum(s_tot[:], s_g[:], axis=mybir.AxisListType.X)
                inv_s = spool.tile([B, 1], F32, tag="invs")
                nc.vector.reciprocal(inv_s[:], s_tot[:])
                x_sb = spool.tile([B, 512], F32, tag="x")
                nc.vector.tensor_scalar(
                    out=x_sb[:], in0=ysg[:], scalar1=inv_s[:], scalar2=None,
                    op0=OP.mult,
                )
                tpx = ps_tp.tile([128, 512], F32, tag="tp")
                for k in range(4):
                    nc.tensor.transpose(
                        tpx[:, k * 32 : (k + 1) * 32],
                        x_sb[:, k * 128 : (k + 1) * 128],
                        ident_sb[0:32, 0:32],
                    )
                xT = spool.tile([128, 128], BF16, tag="xT", bufs=3)
                nc.scalar.copy(xT[:], tpx[:, 0:128])

                h_prev = h_new

    nc.compile()
    return nc


def _host_rng():
    """Reproduce the reference's dropout masks and gumbel noise bit-exactly."""
    cpu = jax.devices("cpu")[0]
    with jax.default_device(cpu):
        import jax.numpy as jnp

        keys = jax.random.split(jax.random.key(42), LEN)

        @jax.jit
        def step(k):
            kd, kg = jax.random.split(k)
            mask = jax.random.bernoulli(kd, KEEP, (B, RNN))
            U = jax.random.uniform(kg, (B, VOCAB), jnp.float32)
            G = -jnp.log(-jnp.log(U + EPS) + EPS)
            return mask, G

        masks = np.zeros((LEN, B, RNN), np.float32)
        Gs = np.zeros((LEN, B, VOCAB), np.float32)
        inv_keep = np.float32(1.0) / np.float32(KEEP)
        for t in range(LEN):
            m, G = step(keys[t])
            masks[t] = np.where(np.asarray(m), inv_keep, np.float32(0.0))
            Gs[t] = np.asarray(G)
    return masks, Gs


def _prepare_in_maps(h, inp, embedding, Wx, Wh, b_gru, Wd, bd, gamma, length, steps=LEN):
    h = np.asarray(h, np.float32)
    inp = np.asarray(inp, np.float32)
    embedding = np.asarray(embedding, np.float32)
    Wx = np.asarray(Wx, np.float32)
    Wh = np.asarray(Wh, np.float32)
    b_gru = np.asarray(b_gru, np.float32)
    Wd = np.asarray(Wd, np.float32)
    bd = np.asarray(bd, np.float32)
    gamma_f = np.float32(np.asarray(gamma))
    assert int(length) == LEN, f"kernel compiled for length={LEN}, got {length}"
    del length
    assert not np.any(b_gru), "kernel assumes b_gru == 0 (spec fill: zeros)"
    inv_gamma = np.float32(1.0) / gamma_f

    masks, Gs = _host_rng()

    bf = ml_dtypes.bfloat16

    # ---- shared (replicated) host arrays ----
    Wh_s = Wh.copy()
    Wh_s[:, 1024:1536] *= np.float32(0.5)  # folded into (tanh+1)*hn' sigmoid fusion
    wxh_np = np.ascontiguousarray(
        np.concatenate([Wx.reshape(4, 128, 1536), Wh_s.reshape(4, 128, 1536)], axis=0)
        .transpose(1, 0, 2)
        .reshape(128, 8 * 1536)
        .astype(bf)
    )
    mask_np = np.ascontiguousarray(
        masks.reshape(LEN, B, 4, 128).transpose(0, 3, 2, 1).reshape(LEN, 128, 128)
    )
    h0T_np = np.ascontiguousarray(
        h.reshape(B, 4, 128).transpose(2, 1, 0).reshape(128, 128).astype(bf)
    )
    x0T_np = np.ascontiguousarray(
        inp.reshape(B, 4, 128).transpose(2, 1, 0).reshape(128, 128).astype(bf)
    )
    ident_np = np.eye(128, dtype=np.float32)
    sel_np = np.zeros((128, 32), np.float32)
    sel_np[np.arange(128), np.arange(128) % 32] = 1.0
    gamma_np = np.full((128, 1), gamma_f, np.float32)

    # ---- sharded host arrays ----
    wd_full = np.zeros((RNN, VPAD), np.float32)
    wd_full[:, :VOCAB] = Wd * inv_gamma
    wd_bf = wd_full.astype(bf).reshape(4, 128, NCORES, VC)

    emb_full = np.zeros((VPAD, EMB), np.float32)
    emb_full[:VOCAB] = embedding
    emb_bf = emb_full.astype(bf).reshape(NCORES, NQ, 128, EMB)

    g_full = np.full((LEN, B, VPAD), NEG, np.float32)
    g_full[:, :, :VOCAB] = Gs * inv_gamma + (bd * inv_gamma)[None, None, :]
    g_resh = g_full.reshape(LEN, B, NCORES, 4, GN)

    in_maps = []
    for c in range(NCORES):
        in_maps.append(
            {
                "i_wxh": wxh_np,
                "i_wd": np.ascontiguousarray(
                    wd_bf[:, :, c, :].transpose(1, 0, 2).reshape(128, 4 * VC)
                ),
                "i_emb": np.ascontiguousarray(
                    emb_bf[c].transpose(1, 0, 2).reshape(128, NQ * EMB)
                ),
                "i_g": np.ascontiguousarray(
                    g_resh[:steps, :, c, :, :].transpose(0, 2, 1, 3).reshape(steps, 128, GN)
                ),
                "i_mask": mask_np[:steps],
                "i_h0": h,
                "i_h0T": h0T_np,
                "i_x0T": x0T_np,
                "i_ident": ident_np,
                "i_sel": sel_np,
                "i_gamma": gamma_np,
            }
        )
    return in_maps


def kernel(h, inp, embedding, Wx, Wh, b_gru, Wd, bd, gamma, length):
    in_maps = _prepare_in_maps(
        h, inp, embedding, Wx, Wh, b_gru, Wd, bd, gamma, length, steps=LEN
    )
    if "prog" not in _BUILD_CACHE:
        _BUILD_CACHE["prog"] = _build_program()
    nc = _BUILD_CACHE["prog"]

    res = run_bass_kernel_spmd(nc, in_maps, core_ids=list(range(NCORES)))

    h_seq = res.results[0]["o_hseq"]
    logits = np.empty((B, LEN, VOCAB), np.float32)
    for c in range(NCORES):
        lo = c * VC
        hi = min(lo + VC, VOCAB)
        logits[:, :, lo:hi] = res.results[c]["o_logits"][:, :, : hi - lo]
    return h_seq, logits


# revision 15
# speedup vs baseline: 1.2164x; 1.2164x over previous
"""Trainium2 Bass kernel for nn_DecoderRNN (GRU + gumbel-softmax sampling decoder).

Strategy:
  - Vocab (50257 -> padded 53248) is sharded 8 ways; each core holds its
    Wd / embedding shard resident in SBUF as bf16.
  - Per step: GRU gates via matmuls into a [32, 2048] PSUM (z|r|xn|hn),
    logits via col-tiled matmuls over the vocab shard (4 groups of 1664),
    exp via ScalarE (with free-dim accum for the softmax normalizer),
    PE transposes to reach vocab-major layout, embedding matmul produces
    unnormalized x-partials, and one AllReduce of [y | s] (32x513 f32)
    combines shards.  Division by the global normalizer happens after the
    AllReduce on every core (replicated small state).
  - All RNG (dropout mask + gumbel noise) is precomputed on host with JAX
    threefry (bit-exact vs the reference) and streamed per step.
  - Cross-partition reductions use a selector matmul on PE (elementwise ops
    must be partition-aligned on trn2).
"""

import numpy as np
import ml_dtypes
import jax

import concourse.bacc as bacc
import concourse.bass as bass
import concourse.mybir as mybir
import concourse.tile as tile
from concourse.bass_utils import run_bass_kernel_spmd

F32 = mybir.dt.float32
BF16 = mybir.dt.bfloat16
AF = mybir.ActivationFunctionType
OP = mybir.AluOpType

B, RNN, EMB, VOCAB, LEN = 32, 512, 512, 50257, 64
NCORES = 8
GN = 1664               # vocab columns per PE column-group
VC = 4 * GN             # 6656 vocab columns per core
VPAD = NCORES * VC      # 53248
NQ = VC // 128          # 52 transposed vocab chunks per core
KEEP = 0.9
EPS = 1e-20
NEG = -1.0e30

_BUILD_CACHE = {}


def _build_program(steps=LEN):
    """Build the SPMD Bass program (identical for all cores)."""
    nc = bacc.Bacc("TRN2", target_bir_lowering=False, debug=False, num_devices=NCORES)

    # ---- DRAM I/O ----
    i_wxh = nc.dram_tensor("i_wxh", [128, 8 * 1536], BF16, kind="ExternalInput").ap()
    i_wd = nc.dram_tensor("i_wd", [128, 4 * VC], BF16, kind="ExternalInput").ap()
    i_emb = nc.dram_tensor("i_emb", [128, NQ * 512], BF16, kind="ExternalInput").ap()
    i_g = nc.dram_tensor("i_g", [steps, 128, GN], F32, kind="ExternalInput").ap()
    i_mask = nc.dram_tensor("i_mask", [steps, 128, 128], F32, kind="ExternalInput").ap()
    i_h0 = nc.dram_tensor("i_h0", [B, RNN], F32, kind="ExternalInput").ap()
    i_h0T = nc.dram_tensor("i_h0T", [128, 128], BF16, kind="ExternalInput").ap()
    i_x0T = nc.dram_tensor("i_x0T", [128, 128], BF16, kind="ExternalInput").ap()
    i_ident = nc.dram_tensor("i_ident", [128, 128], F32, kind="ExternalInput").ap()
    i_sel = nc.dram_tensor("i_sel", [128, 32], F32, kind="ExternalInput").ap()
    i_gamma = nc.dram_tensor("i_gamma", [128, 1], F32, kind="ExternalInput").ap()
    o_hseq = nc.dram_tensor("o_hseq", [B, steps, RNN], F32, kind="ExternalOutput").ap()
    o_logits = nc.dram_tensor("o_logits", [B, steps, VC], F32, kind="ExternalOutput").ap()

    with tile.TileContext(nc) as tc:
        with (
            tc.tile_pool(name="const", bufs=1) as cpool,
            tc.tile_pool(name="gbuf", bufs=2) as gpool,
            tc.tile_pool(name="mbuf", bufs=2) as mpool,
            tc.tile_pool(name="work", bufs=2) as wpool,
            tc.tile_pool(name="small", bufs=2) as spool,
            tc.tile_pool(name="ps_big", bufs=1, space="PSUM") as ps_big,
            tc.tile_pool(name="ps_tp", bufs=2, space="PSUM") as ps_tp,
            tc.tile_pool(name="ps_y", bufs=1, space="PSUM") as ps_y,
            tc.tile_pool(name="ps_ys", bufs=1, space="PSUM") as ps_ys,
            tc.tile_pool(name="dram", bufs=2, space="DRAM") as dpool,
        ):
            # ---- resident constants ----
            wxh_sb = cpool.tile([128, 8 * 1536], BF16)
            for q in range(4):
                nc.sync.dma_start(
                    wxh_sb[:, q * 3072 : (q + 1) * 3072],
                    i_wxh[:, q * 3072 : (q + 1) * 3072],
                )
            wd_sb = cpool.tile([128, 4 * VC], BF16)
            for q in range(4):
                nc.sync.dma_start(
                    wd_sb[:, q * VC : (q + 1) * VC], i_wd[:, q * VC : (q + 1) * VC]
                )
            emb_sb = cpool.tile([128, NQ * 512], BF16)
            for q in range(4):
                w = NQ * 512 // 4
                nc.sync.dma_start(
                    emb_sb[:, q * w : (q + 1) * w], i_emb[:, q * w : (q + 1) * w]
                )
            ident_sb = cpool.tile([128, 128], F32)
            nc.sync.dma_start(ident_sb[:], i_ident[:])
            sel_sb = cpool.tile([128, 32], F32)
            nc.sync.dma_start(sel_sb[:], i_sel[:])
            gamma_sb = cpool.tile([128, 1], F32)
            nc.sync.dma_start(gamma_sb[:], i_gamma[:])

            # ---- state tiles (step 0) ----
            h_prev = spool.tile([B, RNN], F32, tag="h", bufs=3)
            nc.sync.dma_start(h_prev[:], i_h0[:])
            hT = spool.tile([128, 128], BF16, tag="hT", bufs=3)
            nc.sync.dma_start(hT[:], i_h0T[:])
            xT = spool.tile([128, 128], BF16, tag="xT", bufs=3)
            nc.sync.dma_start(xT[:], i_x0T[:])

            for t in range(steps):
                last = t == steps - 1
                # -------- stream per-step randomness --------
                m_t = mpool.tile([128, 128], F32, tag="m")
                nc.sync.dma_start(m_t[:], i_mask[t])
                if not last:
                    g_t = gpool.tile([128, GN], F32, tag="g")
                    nc.sync.dma_start(g_t[:], i_g[t])

                # -------- emit h_t (pre-step hidden) --------
                nc.sync.dma_start(o_hseq[:, t, :], h_prev[:])

                # -------- GRU gates: psum cols [z | r | xn | hn] --------
                gp = ps_big.tile([B, 2048], F32, tag="big")
                for k in range(4):  # h-dependent first (ready before x arrives)
                    lhs = hT[:, k * 32 : (k + 1) * 32]
                    wh = wxh_sb[:, (4 + k) * 1536 :]
                    st = k == 0
                    nc.tensor.matmul(gp[:, 0:512], lhs, wh[:, 0:512], start=st, stop=False)
                    nc.tensor.matmul(gp[:, 512:1024], lhs, wh[:, 512:1024], start=st, stop=False)
                    nc.tensor.matmul(
                        gp[:, 1536:2048], lhs, wh[:, 1024:1536], start=st, stop=(k == 3)
                    )
                for k in range(4):
                    lhs = xT[:, k * 32 : (k + 1) * 32]
                    wx = wxh_sb[:, k * 1536 :]
                    sp = k == 3
                    nc.tensor.matmul(gp[:, 0:512], lhs, wx[:, 0:512], start=False, stop=sp)
                    nc.tensor.matmul(gp[:, 512:1024], lhs, wx[:, 512:1024], start=False, stop=sp)
                    nc.tensor.matmul(
                        gp[:, 1024:1536], lhs, wx[:, 1024:1536], start=(k == 0), stop=sp
                    )

                # -------- gate nonlinearities (all partition-aligned @0:32) ----
                # sigmoid(u) = 0.5*tanh(u/2) + 0.5 (keeps ACT in exp_and_others)
                # tanh(u/2) once; sigmoid folded downstream via (t+1) forms
                zr_t = spool.tile([B, 1024], F32, tag="zr")
                nc.scalar.activation(zr_t[:], gp[:, 0:1024], AF.Tanh, scale=0.5)
                # r*hn = (t_r + 1) * hn'   (Wh_n pre-scaled by 0.5 on host)
                rh = spool.tile([B, 512], F32, tag="rh")
                nc.vector.scalar_tensor_tensor(
                    out=rh[:], in0=zr_t[:, 512:1024], scalar=1.0,
                    in1=gp[:, 1536:2048], op0=OP.add, op1=OP.mult,
                )
                nc.vector.tensor_tensor(
                    out=rh[:], in0=rh[:], in1=gp[:, 1024:1536], op=OP.add
                )
                n_t = rh
                nc.scalar.activation(n_t[:], rh[:], AF.Tanh)
                # h_new = n + z*(h-n) = n + 0.5*((t_z+1)*(h-n)); DVE/GPSIMD halves
                d_t = spool.tile([B, 512], F32, tag="d")
                h_new = spool.tile([B, 512], F32, tag="h", bufs=3)
                HH = 256
                nc.vector.tensor_tensor(
                    out=d_t[:, 0:HH], in0=h_prev[:, 0:HH], in1=n_t[:, 0:HH], op=OP.subtract
                )
                nc.gpsimd.tensor_tensor(
                    out=d_t[:, HH:512], in0=h_prev[:, HH:512], in1=n_t[:, HH:512],
                    op=OP.subtract,
                )
                nc.vector.scalar_tensor_tensor(
                    out=d_t[:], in0=zr_t[:, 0:512], scalar=1.0,
                    in1=d_t[:], op0=OP.add, op1=OP.mult,
                )
                nc.vector.scalar_tensor_tensor(
                    out=h_new[:], in0=d_t[:], scalar=0.5,
                    in1=n_t[:], op0=OP.mult, op1=OP.add,
                )

                # -------- transpose h_new; apply dropout mask --------
                tph = ps_tp.tile([128, 512], F32, tag="tp")
                for k in range(4):
                    nc.tensor.transpose(
                        tph[:, k * 32 : (k + 1) * 32],
                        h_new[:, k * 128 : (k + 1) * 128],
                        ident_sb[0:32, 0:32],
                    )
                hT = spool.tile([128, 128], BF16, tag="hT", bufs=3)
                nc.scalar.copy(hT[:], tph[:, 0:128])
                outT = spool.tile([128, 128], BF16, tag="outT")
                nc.vector.tensor_tensor(
                    out=outT[:], in0=tph[:, 0:128], in1=m_t[:], op=OP.mult
                )

                # -------- logits matmuls (col-tiled over 4 vocab groups) -----
                lp = ps_big.tile([128, 2048], F32, tag="big")
                for off, nn_ in ((0, 512), (512, 512), (1024, 512), (1536, 128)):
                    for k in range(4):
                        for j in range(4):
                            nc.tensor.matmul(
                                lp[32 * j : 32 * (j + 1), off : off + nn_],
                                outT[:, k * 32 : (k + 1) * 32],
                                wd_sb[:, k * VC + j * GN + off : k * VC + j * GN + off + nn_],
                                start=(k == 0), stop=(k == 3),
                                tile_position=(0, 32 * j),
                                skip_group_check=True,
                            )

                # -------- softmax numerator first (keeps ACT critical path
                # short); logits output copy happens after, off critical path.
                logits_sb = wpool.tile([128, GN], F32, tag="lsb", bufs=1)
                if not last:
                    exp_sb = wpool.tile([128, GN], F32, tag="exp", bufs=1)
                    s_parts = spool.tile([128, 4], F32, tag="sp")
                    for si, (off, nn_) in enumerate(((0, 512), (512, 512), (1024, 640))):
                        nc.vector.tensor_tensor(
                            out=g_t[:, off : off + nn_],
                            in0=lp[:, off : off + nn_],
                            in1=g_t[:, off : off + nn_],
                            op=OP.add,
                        )
                        nc.scalar.activation(
                            exp_sb[:, off : off + nn_],
                            g_t[:, off : off + nn_],
                            AF.Exp,
                            accum_out=s_parts[:, si : si + 1],
                        )
                # -------- logits output: psum * gamma -> DRAM --------
                nc.scalar.activation(
                    logits_sb[:], lp[:, 0:GN], AF.Copy, bias=0.0, scale=gamma_sb[:, 0:1]
                )
                for j in range(4):
                    nc.sync.dma_start(
                        o_logits[:, t, j * GN : (j + 1) * GN],
                        logits_sb[32 * j : 32 * (j + 1), :],
                    )

                if last:
                    h_prev = h_new
                    continue

                # -------- transpose exp to vocab-major (13 chunks of 128) ----
                expT = wpool.tile([128, NQ * 32], BF16, tag="expT", bufs=1)
                for r in range(4):
                    cs = range(4 * r, min(4 * r + 4, 13))
                    nch = len(cs)
                    te = ps_tp.tile([128, 512], F32, tag="tp")
                    for ci, c in enumerate(cs):
                        nc.tensor.transpose(
                            te[:, ci * 128 : (ci + 1) * 128],
                            exp_sb[:, c * 128 : (c + 1) * 128],
                            ident_sb[:],
                        )
                    src = te[:, 0 : nch * 128].rearrange(
                        "p (c j bb) -> p c j bb", c=nch, j=4, bb=32
                    )
                    dst = expT[:].rearrange("p (j c bb) -> p c j bb", j=4, c=13, bb=32)[
                        :, 4 * r : 4 * r + nch, :, :
                    ]
                    if r % 2 == 0:
                        nc.vector.tensor_copy(dst, src)
                    else:
                        nc.scalar.copy(dst, src)

                # -------- embedding matmul (partials per column-group) -------
                yp = ps_y.tile([128, 512], F32, tag="yp")
                for c in range(13):
                    for j in range(4):
                        q = 13 * j + c
                        nc.tensor.matmul(
                            yp[32 * j : 32 * (j + 1), :],
                            expT[:, q * 32 : (q + 1) * 32],
                            emb_sb[:, q * 512 : (q + 1) * 512],
                            start=(c == 0), stop=(c == 12),
                            tile_position=(0, 32 * j),
                            skip_group_check=True,
                        )

                # -------- fold partials across partition blocks (PE) --------
                y_sb = spool.tile([128, 512], F32, tag="ysb")
                nc.scalar.copy(y_sb[:], yp[:])
                ys = ps_ys.tile([B, 512], F32, tag="ys")
                nc.tensor.matmul(ys[:], sel_sb[:], y_sb[:], start=True, stop=True)
                xchg = spool.tile([B, 513], F32, tag="xchg")
                nc.vector.tensor_copy(xchg[:, 0:512], ys[:])
                # s: gather the 128x3 partials into batch-partition layout
                s_gather = spool.tile([B, 12], F32, tag="sg")
                for j in range(4):
                    nc.sync.dma_start(
                        s_gather[:, j * 3 : (j + 1) * 3],
                        s_parts[32 * j : 32 * (j + 1), 0:3],
                    )
                nc.vector.reduce_sum(
                    xchg[:, 512:513], s_gather[:], axis=mybir.AxisListType.X
                )

                # -------- AllReduce of [y | s] --------
                cc_in = dpool.tile([B, 513], F32, tag="ccin")
                cc_out = dpool.tile([B, 513], F32, tag="ccout", addr_space="Shared")
                nc.sync.dma_start(cc_in[:], xchg[:])
                nc.gpsimd.collective_compute(
                    "AllReduce",
                    OP.add,
                    replica_groups=[list(range(NCORES))],
                    ins=[cc_in[:]],
                    outs=[cc_out[:]],
                )
                gy = spool.tile([B, 513], F32, tag="gy")
                nc.sync.dma_start(gy[:], cc_out[:])
                inv_s = spool.tile([B, 1], F32, tag="invs")
                nc.vector.reciprocal(inv_s[:], gy[:, 512:513])
                x_sb = gy
                nc.vector.tensor_scalar(
                    out=x_sb[:, 0:512], in0=gy[:, 0:512], scalar1=inv_s[:],
                    scalar2=None, op0=OP.mult,
                )
                tpx = ps_tp.tile([128, 512], F32, tag="tp")
                for k in range(4):
                    nc.tensor.transpose(
                        tpx[:, k * 32 : (k + 1) * 32],
                        x_sb[:, k * 128 : (k + 1) * 128],
                        ident_sb[0:32, 0:32],
                    )
                xT = spool.tile([128, 128], BF16, tag="xT", bufs=3)
                nc.scalar.copy(xT[:], tpx[:, 0:128])

                h_prev = h_new

    nc.compile()
    return nc


def _host_rng():
    """Reproduce the reference's dropout masks and gumbel noise bit-exactly."""
    cpu = jax.devices("cpu")[0]
    with jax.default_device(cpu):
        import jax.numpy as jnp

        keys = jax.random.split(jax.random.key(42), LEN)

        @jax.jit
        def step(k):
            kd, kg = jax.random.split(k)
            mask = jax.random.bernoulli(kd, KEEP, (B, RNN))
            U = jax.random.uniform(kg, (B, VOCAB), jnp.float32)
            G = -jnp.log(-jnp.log(U + EPS) + EPS)
            return mask, G

        masks = np.zeros((LEN, B, RNN), np.float32)
        Gs = np.zeros((LEN, B, VOCAB), np.float32)
        inv_keep = np.float32(1.0) / np.float32(KEEP)
        for t in range(LEN):
            m, G = step(keys[t])
            masks[t] = np.where(np.asarray(m), inv_keep, np.float32(0.0))
            Gs[t] = np.asarray(G)
    return masks, Gs


def _prepare_in_maps(h, inp, embedding, Wx, Wh, b_gru, Wd, bd, gamma, length, steps=LEN):
    h = np.asarray(h, np.float32)
    inp = np.asarray(inp, np.float32)
    embedding = np.asarray(embedding, np.float32)
    Wx = np.asarray(Wx, np.float32)
    Wh = np.asarray(Wh, np.float32)
    b_gru = np.asarray(b_gru, np.float32)
    Wd = np.asarray(Wd, np.float32)
    bd = np.asarray(bd, np.float32)
    gamma_f = np.float32(np.asarray(gamma))
    assert int(length) == LEN, f"kernel compiled for length={LEN}, got {length}"
    del length
    assert not np.any(b_gru), "kernel assumes b_gru == 0 (spec fill: zeros)"
    inv_gamma = np.float32(1.0) / gamma_f

    masks, Gs = _host_rng()

    bf = ml_dtypes.bfloat16

    # ---- shared (replicated) host arrays ----
    Wh_s = Wh.copy()
    Wh_s[:, 1024:1536] *= np.float32(0.5)  # folded into (tanh+1)*hn' sigmoid fusion
    wxh_np = np.ascontiguousarray(
        np.concatenate([Wx.reshape(4, 128, 1536), Wh_s.reshape(4, 128, 1536)], axis=0)
        .transpose(1, 0, 2)
        .reshape(128, 8 * 1536)
        .astype(bf)
    )
    mask_np = np.ascontiguousarray(
        masks.reshape(LEN, B, 4, 128).transpose(0, 3, 2, 1).reshape(LEN, 128, 128)
    )
    h0T_np = np.ascontiguousarray(
        h.reshape(B, 4, 128).transpose(2, 1, 0).reshape(128, 128).astype(bf)
    )
    x0T_np = np.ascontiguousarray(
        inp.reshape(B, 4, 128).transpose(2, 1, 0).reshape(128, 128).astype(bf)
    )
    ident_np = np.eye(128, dtype=np.float32)
    sel_np = np.zeros((128, 32), np.float32)
    sel_np[np.arange(128), np.arange(128) % 32] = 1.0
    gamma_np = np.full((128, 1), gamma_f, np.float32)

    # ---- sharded host arrays ----
    wd_full = np.zeros((RNN, VPAD), np.float32)
    wd_full[:, :VOCAB] = Wd * inv_gamma
    wd_bf = wd_full.astype(bf).reshape(4, 128, NCORES, VC)

    emb_full = np.zeros((VPAD, EMB), np.float32)
    emb_full[:VOCAB] = embedding
    emb_bf = emb_full.astype(bf).reshape(NCORES, NQ, 128, EMB)

    g_full = np.full((LEN, B, VPAD), NEG, np.float32)
    g_full[:, :, :VOCAB] = Gs * inv_gamma + (bd * inv_gamma)[None, None, :]
    g_resh = g_full.reshape(LEN, B, NCORES, 4, GN)

    in_maps = []
    for c in range(NCORES):
        in_maps.append(
            {
                "i_wxh": wxh_np,
                "i_wd": np.ascontiguousarray(
                    wd_bf[:, :, c, :].transpose(1, 0, 2).reshape(128, 4 * VC)
                ),
                "i_emb": np.ascontiguousarray(
                    emb_bf[c].transpose(1, 0, 2).reshape(128, NQ * EMB)
                ),
                "i_g": np.ascontiguousarray(
                    g_resh[:steps, :, c, :, :].transpose(0, 2, 1, 3).reshape(steps, 128, GN)
                ),
                "i_mask": mask_np[:steps],
                "i_h0": h,
                "i_h0T": h0T_np,
                "i_x0T": x0T_np,
                "i_ident": ident_np,
                "i_sel": sel_np,
                "i_gamma": gamma_np,
            }
        )
    return in_maps


def kernel(h, inp, embedding, Wx, Wh, b_gru, Wd, bd, gamma, length):
    in_maps = _prepare_in_maps(
        h, inp, embedding, Wx, Wh, b_gru, Wd, bd, gamma, length, steps=LEN
    )
    if "prog" not in _BUILD_CACHE:
        _BUILD_CACHE["prog"] = _build_program()
    nc = _BUILD_CACHE["prog"]

    res = run_bass_kernel_spmd(nc, in_maps, core_ids=list(range(NCORES)))

    h_seq = res.results[0]["o_hseq"]
    logits = np.empty((B, LEN, VOCAB), np.float32)
    for c in range(NCORES):
        lo = c * VC
        hi = min(lo + VC, VOCAB)
        logits[:, :, lo:hi] = res.results[c]["o_logits"][:, :, : hi - lo]
    return h_seq, logits
